# revision 2
# baseline (speedup 1.0000x reference)
"""2D DCT-II (4096x4096, fp32) on 8 TRN2 NeuronCores.

out = C0 @ x @ C1^T with C0 = C1 = C, C[k, i] = cos(pi*(2i+1)*k/(2N)).

Strategy: each core c computes a 512-row block of the output directly:
    out[512c:512(c+1), :] = (C[512c:512(c+1), :] @ x) @ C^T
No inter-core communication needed; per-core work is two chained
[512,4096]x[4096,4096] matmuls (34.4 GFLOP) and ~144 MB of HBM traffic
(x 64MB + C^T 64MB + slices) -> balanced at the compute/memory ridge.

Tensor-engine mapping (fp32r = full-rate FP22 mode at free-dim >= 256):
  stage 1 computes T^T[n, m] = sum_k x[k, n] * C^T[k, 512c+m]
     lhsT (stationary) = x tile [K=128, M=128] (natural x layout)
     rhs  (moving)     = C^T slice tile [K=128, N=512]
  so the intermediate lands transposed in SBUF, which is exactly the
  layout stage 2 needs as its stationary operand:
  stage 2 computes out[m, v] = sum_j T^T[j, m] * C^T[j, v]
     lhsT = T^T tile [128, 128] (in SBUF), rhs = C^T tile [128, 512]
"""

import math

import numpy as np

import concourse.mybir as mybir
import concourse.tile as tile
from concourse import bacc
from concourse.bass_utils import run_bass_kernel_spmd

N = 4096
P = 128
KT = N // P  # 32 k-tiles
NCORES = 8
RB = N // NCORES  # 512 output rows per core

f32 = mybir.dt.float32
f32r = mybir.dt.float32r

_CACHE = {}


def _build():
    nc = bacc.Bacc("TRN2", target_bir_lowering=False, debug=False)
    x_d = nc.dram_tensor("x", [N, N], f32r, kind="ExternalInput")
    c0t_d = nc.dram_tensor("c0t", [N, RB], f32r, kind="ExternalInput")
    c1t_d = nc.dram_tensor("c1t", [N, N], f32r, kind="ExternalInput")
    out_d = nc.dram_tensor("out", [RB, N], f32, kind="ExternalOutput")

    with tile.TileContext(nc) as tc:
        with (
            tc.tile_pool(name="persist", bufs=1) as persist,
            tc.tile_pool(name="xin", bufs=4) as xin,
            tc.tile_pool(name="cin", bufs=4) as cin,
            tc.tile_pool(name="osb", bufs=4) as osb,
            tc.tile_pool(name="ps", bufs=1, space="PSUM") as ps,
        ):
            # C^T slice for this core's rows: [k, m] as [128, 32, 512]
            c0t_sb = persist.tile([P, KT, RB], f32r, tag="c0t", name="c0t_sb")
            for k in range(KT):
                nc.sync.dma_start(c0t_sb[:, k, :], c0t_d[k * P:(k + 1) * P, :])
            # intermediate T^T: [n, m] as [128, 32, 512]
            t_sb = persist.tile([P, KT, RB], f32r, tag="tsb", name="t_sb")

            # ---- stage 1: T^T[n, m] = sum_k x[k, n] * c0t[k, m] ----
            for ngrp in range(4):
                psums = [ps.tile([P, RB], f32, tag=f"ps{i}", name=f"ps{i}") for i in range(8)]
                for k in range(KT):
                    xt = xin.tile([P, 8 * P], f32r, tag="xt", name="xt")
                    nc.sync.dma_start(
                        xt[:],
                        x_d[k * P:(k + 1) * P, ngrp * 1024:(ngrp + 1) * 1024],
                    )
                    for nb in range(8):
                        nc.tensor.matmul(
                            psums[nb][:],
                            xt[:, nb * P:(nb + 1) * P],
                            c0t_sb[:, k, :],
                            start=(k == 0),
                            stop=(k == KT - 1),
                        )
                for nb in range(8):
                    nc.vector.tensor_copy(
                        t_sb[:, ngrp * 8 + nb, :], psums[nb][:]
                    )

            # ---- stage 2: out[m, v] = sum_j T^T[j, m] * c1t[j, v] ----
            for vgrp in range(4):
                psums = [ps.tile([P, RB], f32, tag=f"ps{i}", name=f"ps{i}") for i in range(8)]
                for j in range(KT):
                    ct = cin.tile([P, 8 * P], f32r, tag="ct", name="ct")
                    nc.sync.dma_start(
                        ct[:],
                        c1t_d[j * P:(j + 1) * P, vgrp * 1024:(vgrp + 1) * 1024],
                    )
                    for mb in range(4):
                        for v2 in range(2):
                            nc.tensor.matmul(
                                psums[mb * 2 + v2][:],
                                t_sb[:, j, mb * P:(mb + 1) * P],
                                ct[:, v2 * RB:(v2 + 1) * RB],
                                start=(j == 0),
                                stop=(j == KT - 1),
                            )
                for mb in range(4):
                    for v2 in range(2):
                        ot = osb.tile([P, RB], f32, tag="ot", name="ot")
                        nc.vector.tensor_copy(ot[:], psums[mb * 2 + v2][:])
                        nc.sync.dma_start(
                            out_d[
                                mb * P:(mb + 1) * P,
                                vgrp * 1024 + v2 * RB:
                                vgrp * 1024 + (v2 + 1) * RB,
                            ],
                            ot[:],
                        )
    nc.compile()
    return nc


def _get_nc():
    if "nc" not in _CACHE:
        _CACHE["nc"] = _build()
    return _CACHE["nc"]


def _dct_basis_t():
    """C^T as float32 [N, N]: C^T[i, k] = cos(pi*(2i+1)*k/(2N)).

    Matches the reference's float32 jnp computation (fp32 argument
    arithmetic) so basis rounding does not diverge from the oracle."""
    if "ct" in _CACHE:
        return _CACHE["ct"]
    ct = None
    try:
        import jax
        import jax.numpy as jnp

        cpus = jax.devices("cpu")
        with jax.default_device(cpus[0]):
            k = jnp.arange(N, dtype=jnp.float32)[:, None]
            i = jnp.arange(N, dtype=jnp.float32)[None, :]
            c = jnp.cos((jnp.pi / (2.0 * N)) * (2.0 * i + 1.0) * k)
            ct = np.ascontiguousarray(np.asarray(c).T)
    except Exception:
        pass
    if ct is None:
        k = np.arange(N, dtype=np.float32)[:, None]
        i = np.arange(N, dtype=np.float32)[None, :]
        s = math.pi / (2.0 * N)
        arg = (s * (2.0 * i + 1.0)).astype(np.float32) * k
        ct = np.ascontiguousarray(np.cos(arg.astype(np.float32)).T)
    _CACHE["ct"] = ct
    return ct


def _in_maps(x):
    x = np.ascontiguousarray(np.asarray(x, dtype=np.float32))
    ct = _dct_basis_t()
    return [
        {
            "x": x,
            "c0t": np.ascontiguousarray(ct[:, c * RB:(c + 1) * RB]),
            "c1t": ct,
        }
        for c in range(NCORES)
    ]


def _run(x, **kwargs):
    nc = _get_nc()
    res = run_bass_kernel_spmd(
        nc, _in_maps(x), core_ids=list(range(NCORES)), **kwargs
    )
    out = np.concatenate(
        [res.results[c]["out"] for c in range(NCORES)], axis=0
    )
    return out, res


def kernel(x):
    out, _ = _run(x)
    return out


# revision 3
# speedup vs baseline: 1.1497x; 1.1497x over previous
"""2D DCT-II (4096x4096, fp32) on 8 TRN2 NeuronCores.

out = C0 @ x @ C1^T with C0 = C1 = C, C[k, i] = cos(pi*(2i+1)*k/(2N)).

Strategy: each core c computes a 512-row block of the output directly:
    out[512c:512(c+1), :] = (C[512c:512(c+1), :] @ x) @ C^T
No inter-core communication; per-core work is two chained
[512,4096]x[4096,4096] matmuls (34.4 GFLOP at fp32r full PE rate
~ 437 us) and ~144 MB of HBM traffic (~400 us at 358 GB/s) -> the
compute/memory ridge, PE-bound by a small margin.

Tensor-engine mapping (fp32r = full-rate FP22 mode at free-dim >= 256):
  stage 1: T^T[n, m] = sum_k x[k, n] * C^T[k, 512c+m]
     lhsT (stationary) = x tile [K=128, M=128] (natural x layout)
     rhs  (moving)     = C^T slice tile [K=128, N=512]
  -> intermediate lands transposed in SBUF, exactly the layout stage 2
     needs as its stationary operand:
  stage 2: out[m, v] = sum_j T^T[j, m] * C^T[j, v]
     lhsT = T^T tile [128, 128] (SBUF-resident), rhs = C^T tile [128, 512]

Pipelining: output columns are processed in 512-wide groups; consecutive
groups alternate between two 4-bank PSUM sets so one group's PSUM->SBUF
drain overlaps the next group's matmuls. Streaming operands (x, C^T)
arrive as 1 MB DMAs (4 k-tiles x 512 cols) triple-buffered.
"""

import math

import numpy as np

import concourse.mybir as mybir
import concourse.tile as tile
from concourse import bacc
from concourse.bass_utils import run_bass_kernel_spmd

N = 4096
P = 128
KT = N // P  # 32 k-tiles
NCORES = 8
RB = N // NCORES  # 512 output rows per core
G = 512  # column-group width
NG = N // G  # 8 groups
KQ = 4  # k-tiles per streaming DMA (1 MB)

f32 = mybir.dt.float32
f32r = mybir.dt.float32r

_CACHE = {}


def _build():
    nc = bacc.Bacc("TRN2", target_bir_lowering=False, debug=False)
    x_d = nc.dram_tensor("x", [N, N], f32r, kind="ExternalInput")
    c0t_d = nc.dram_tensor("c0t", [N, RB], f32r, kind="ExternalInput")
    c1t_d = nc.dram_tensor("c1t", [N, N], f32r, kind="ExternalInput")
    out_d = nc.dram_tensor("out", [RB, N], f32, kind="ExternalOutput")

    with tile.TileContext(nc) as tc:
        with (
            tc.tile_pool(name="persist", bufs=1) as persist,
            tc.tile_pool(name="xin", bufs=3) as xin,
            tc.tile_pool(name="cin", bufs=3) as cin,
            tc.tile_pool(name="osb", bufs=8) as osb,
            tc.tile_pool(name="ps", bufs=1, space="PSUM") as ps,
        ):
            # C^T slice for this core's rows: [k, m] as [128, 32, 512]
            c0t_sb = persist.tile([P, KT, RB], f32r, tag="c0t", name="c0t_sb")
            # intermediate T^T: [n, m] as [128, 32, 512]
            t_sb = persist.tile([P, KT, RB], f32r, tag="tsb", name="t_sb")

            def psbank(g, i):
                return ps.tile(
                    [P, G], f32, tag=f"ps{(g % 2) * 4 + i}",
                    name=f"ps{(g % 2) * 4 + i}",
                )

            # ---- stage 1: T^T[n, m] = sum_k x[k, n] * c0t[k, m] ----
            # column group g covers n in [g*512, (g+1)*512) = 4 n-blocks
            for g in range(NG):
                banks = [psbank(g, i) for i in range(4)]
                for kq in range(KT // KQ):
                    if g == 0:
                        # interleave the c0t preload with the first group's
                        # x streaming so the PE isn't starved at startup
                        nc.sync.dma_start(
                            c0t_sb[:, kq * KQ:(kq + 1) * KQ, :],
                            c0t_d[kq * KQ * P:(kq + 1) * KQ * P, :].rearrange(
                                "(o p) m -> p o m", p=P
                            ),
                        )
                    xt = xin.tile([P, KQ, G], f32r, tag="xt", name="xt")
                    nc.sync.dma_start(
                        xt[:],
                        x_d[
                            kq * KQ * P:(kq + 1) * KQ * P, g * G:(g + 1) * G
                        ].rearrange("(o p) n -> p o n", p=P),
                    )
                    for ko in range(KQ):
                        k = kq * KQ + ko
                        for nb in range(4):
                            nc.tensor.matmul(
                                banks[nb][:],
                                xt[:, ko, nb * P:(nb + 1) * P],
                                c0t_sb[:, k, :],
                                start=(k == 0),
                                stop=(k == KT - 1),
                            )
                for nb in range(4):
                    nc.vector.tensor_copy(
                        t_sb[:, g * 4 + nb, :], banks[nb][:]
                    )

            # ---- stage 2: out[m, v] = sum_j T^T[j, m] * c1t[j, v] ----
            # column group g covers v in [g*512, (g+1)*512); 4 m-tiles
            for g in range(NG):
                banks = [psbank(g, i) for i in range(4)]
                for jq in range(KT // KQ):
                    ct = cin.tile([P, KQ, G], f32r, tag="ct", name="ct")
                    nc.sync.dma_start(
                        ct[:],
                        c1t_d[
                            jq * KQ * P:(jq + 1) * KQ * P, g * G:(g + 1) * G
                        ].rearrange("(o p) v -> p o v", p=P),
                    )
                    for jo in range(KQ):
                        j = jq * KQ + jo
                        for mb in range(4):
                            nc.tensor.matmul(
                                banks[mb][:],
                                t_sb[:, j, mb * P:(mb + 1) * P],
                                ct[:, jo, :],
                                start=(j == 0),
                                stop=(j == KT - 1),
                            )
                for mb in range(4):
                    ot = osb.tile([P, G], f32, tag="ot", name="ot")
                    nc.vector.tensor_copy(ot[:], banks[mb][:])
                    nc.sync.dma_start(
                        out_d[mb * P:(mb + 1) * P, g * G:(g + 1) * G],
                        ot[:],
                    )
    nc.compile()
    return nc


def _get_nc():
    if "nc" not in _CACHE:
        _CACHE["nc"] = _build()
    return _CACHE["nc"]


def _dct_basis_t():
    """C^T as float32 [N, N]: C^T[i, k] = cos(pi*(2i+1)*k/(2N)).

    Matches the reference's float32 jnp computation (fp32 argument
    arithmetic) so basis rounding does not diverge from the oracle."""
    if "ct" in _CACHE:
        return _CACHE["ct"]
    ct = None
    try:
        import jax
        import jax.numpy as jnp

        cpus = jax.devices("cpu")
        with jax.default_device(cpus[0]):
            k = jnp.arange(N, dtype=jnp.float32)[:, None]
            i = jnp.arange(N, dtype=jnp.float32)[None, :]
            c = jnp.cos((jnp.pi / (2.0 * N)) * (2.0 * i + 1.0) * k)
            ct = np.ascontiguousarray(np.asarray(c).T)
    except Exception:
        pass
    if ct is None:
        k = np.arange(N, dtype=np.float32)[:, None]
        i = np.arange(N, dtype=np.float32)[None, :]
        s = math.pi / (2.0 * N)
        arg = (s * (2.0 * i + 1.0)).astype(np.float32) * k
        ct = np.ascontiguousarray(np.cos(arg.astype(np.float32)).T)
    _CACHE["ct"] = ct
    return ct


def _in_maps(x):
    x = np.ascontiguousarray(np.asarray(x, dtype=np.float32))
    ct = _dct_basis_t()
    return [
        {
            "x": x,
            "c0t": np.ascontiguousarray(ct[:, c * RB:(c + 1) * RB]),
            "c1t": ct,
        }
        for c in range(NCORES)
    ]


def _run(x, **kwargs):
    nc = _get_nc()
    res = run_bass_kernel_spmd(
        nc, _in_maps(x), core_ids=list(range(NCORES)), **kwargs
    )
    out = np.concatenate(
        [res.results[c]["out"] for c in range(NCORES)], axis=0
    )
    return out, res


def kernel(x):
    out, _ = _run(x)
    return out


# revision 5
# speedup vs baseline: 2.0514x; 1.7842x over previous
"""2D DCT-II (4096x4096, fp32) on 8 TRN2 NeuronCores.

out = C0 @ x @ C1^T with C0 = C1 = C, C[k, i] = cos(pi*(2i+1)*k/(2N)).

The DCT-II basis has the reflection symmetry
    C[u, N-1-i] = (-1)^u * C[u, i]
which lets both 1D transforms be folded to half-length contractions by
splitting outputs on parity ("even/odd decomposition" of a fast DCT):

  stage 1 (rows):  T[u, j]   = sum_{i<N/2} C[u, i] * (x[i,j] +- x[N-1-i,j])
  stage 2 (cols):  out[u, v] = sum_{j<N/2} C[v, j] * (T[u,j] +- T[u,N-1-j])

All folds of x happen on the HOST (cheap numpy adds), so the device does
half the FLOPs and half the HBM traffic of the naive separable DCT:

  - cores 0-3 own the even output rows u, cores 4-7 the odd rows
    (1024-row blocks, 512 rows each after parity-split);
  - per core, host supplies xa/xb = doubly-folded x quarters [2048,2048]
    (for even-v / odd-v outputs), c0tp = C^T[:2048, u-slice] [2048,512],
    and c1teo = parity-packed C^T[:2048] columns [2048,4096];
  - stage 1: TE^T[j',m] = sum_i' xa[i',j'] * c0tp[i',m] (and TO from xb)
    via fp32r matmuls, lhsT = xa tile (stationary), rhs = c0tp -> the
    intermediates land transposed in SBUF, which is the exact stationary
    layout stage 2 needs;
  - stage 2: out[m, 2v'+p] = sum_j' (TE|TO)^T[j',m] * c1t_p[j',v'],
    even/odd results interleaved into [128,1024] SBUF tiles by strided
    DVE copies, stored with SWDGE so output stores never block the
    HWDGE load FIFO.

PSUM pipelining: 4-bank accumulation groups alternate between two bank
sets so a group's drain overlaps the next group's matmuls. Streaming
operands arrive as 1 MB DMAs (4 k-tiles x 512 cols), triple-buffered.

Per-core: 1024 matmuls (128x128x512 fp32r, ~227 ns each ~ 232 us) and
~76 MB HBM (~220 us) -> PE-bound just above the ridge.
"""

import math

import numpy as np

import concourse.mybir as mybir
import concourse.tile as tile
from concourse import bacc
from concourse.bass_utils import run_bass_kernel_spmd

N = 4096
H = N // 2  # 2048, folded contraction length
P = 128
HT = H // P  # 16 k-tiles per folded contraction
NCORES = 8
RB = 512  # output rows per core
G = 512  # column-group width
KQ = 4  # k-tiles per streaming DMA (1 MB)

f32 = mybir.dt.float32
f32r = mybir.dt.float32r

_CACHE = {}


def _build():
    nc = bacc.Bacc("TRN2", target_bir_lowering=False, debug=False)
    xa_d = nc.dram_tensor("xa", [H, H], f32r, kind="ExternalInput")
    xb_d = nc.dram_tensor("xb", [H, H], f32r, kind="ExternalInput")
    c0tp_d = nc.dram_tensor("c0tp", [H, RB], f32r, kind="ExternalInput")
    c1teo_d = nc.dram_tensor("c1teo", [H, N], f32r, kind="ExternalInput")
    out_d = nc.dram_tensor("out", [RB, N], f32, kind="ExternalOutput")

    ggc = 0  # global accumulation-group counter (A/B PSUM set parity)

    with tile.TileContext(nc) as tc:
        with (
            tc.tile_pool(name="persist", bufs=1) as persist,
            tc.tile_pool(name="xin", bufs=3) as xin,
            tc.tile_pool(name="cin", bufs=3) as cin,
            tc.tile_pool(name="osb", bufs=1) as osb,
            tc.tile_pool(name="ps", bufs=1, space="PSUM") as ps,
        ):
            # C^T u-slice for this core's rows: [i', m] as [128, 16, 512]
            c0tp_sb = persist.tile([P, HT, RB], f32r, tag="c0", name="c0tp_sb")
            # folded intermediates TE^T / TO^T: [j', m] as [128, 16, 512]
            t_sb = [
                persist.tile([P, HT, RB], f32r, tag=f"t{h}", name=f"t{h}_sb")
                for h in range(2)
            ]

            def banks(g):
                return [
                    ps.tile(
                        [P, G], f32, tag=f"ps{(g % 2) * 4 + i}",
                        name=f"ps{(g % 2) * 4 + i}",
                    )
                    for i in range(4)
                ]

            # ---- stage 1: T(E|O)^T[j', m] = sum_i' x(a|b)[i', j'] * c0tp[i', m]
            for h in range(2):
                src = xa_d if h == 0 else xb_d
                for g in range(4):  # j'-column groups of 512
                    bk = banks(ggc)
                    ggc += 1
                    for kq in range(HT // KQ):
                        if h == 0 and g == 0 and kq == 0:
                            # fine-grained first chunk: let the first
                            # matmuls start after ~512 KB instead of 2 MB
                            for ko in range(KQ):
                                nc.sync.dma_start(
                                    c0tp_sb[:, ko, :],
                                    c0tp_d[ko * P:(ko + 1) * P, :],
                                )
                                if ko == 0:
                                    xt = xin.tile(
                                        [P, KQ, G], f32r, tag="xt", name="xt"
                                    )
                                nc.sync.dma_start(
                                    xt[:, ko, :],
                                    src[ko * P:(ko + 1) * P, 0:G],
                                )
                        else:
                            if h == 0 and g == 0:
                                nc.sync.dma_start(
                                    c0tp_sb[:, kq * KQ:(kq + 1) * KQ, :],
                                    c0tp_d[
                                        kq * KQ * P:(kq + 1) * KQ * P, :
                                    ].rearrange("(o p) m -> p o m", p=P),
                                )
                            xt = xin.tile([P, KQ, G], f32r, tag="xt", name="xt")
                            nc.sync.dma_start(
                                xt[:],
                                src[
                                    kq * KQ * P:(kq + 1) * KQ * P,
                                    g * G:(g + 1) * G,
                                ].rearrange("(o p) n -> p o n", p=P),
                            )
                        for ko in range(KQ):
                            it = kq * KQ + ko
                            for jb in range(4):
                                nc.tensor.matmul(
                                    bk[jb][:],
                                    xt[:, ko, jb * P:(jb + 1) * P],
                                    c0tp_sb[:, it, :],
                                    start=(it == 0),
                                    stop=(it == HT - 1),
                                )
                    for jb in range(4):
                        nc.vector.tensor_copy(
                            t_sb[h][:, g * 4 + jb, :], bk[jb][:]
                        )

            # ---- stage 2: out[m, 2v'+p] = sum_j' T(E|O)^T[j',m] * c1t_p[j',v']
            for vg in range(4):  # 1024-wide output column groups
                outs = [
                    osb.tile(
                        [P, 2 * G], f32, tag=f"ot{mb}", name=f"ot{mb}",
                        bufs=2,
                    )
                    for mb in range(4)
                ]
                for par in range(2):  # even v then odd v
                    bk = banks(ggc)
                    ggc += 1
                    for jq in range(HT // KQ):
                        ct = cin.tile([P, KQ, G], f32r, tag="ct", name="ct")
                        col0 = (2 * vg + par) * G
                        nc.sync.dma_start(
                            ct[:],
                            c1teo_d[
                                jq * KQ * P:(jq + 1) * KQ * P,
                                col0:col0 + G,
                            ].rearrange("(o p) v -> p o v", p=P),
                        )
                        for jo in range(KQ):
                            jt = jq * KQ + jo
                            for mb in range(4):
                                nc.tensor.matmul(
                                    bk[mb][:],
                                    t_sb[par][:, jt, mb * P:(mb + 1) * P],
                                    ct[:, jo, :],
                                    start=(jt == 0),
                                    stop=(jt == HT - 1),
                                )
                    for mb in range(4):
                        # interleave this parity's columns: out[:, par::2]
                        strided = outs[mb].rearrange(
                            "p (v two) -> p v two", two=2
                        )[:, :, par]
                        nc.vector.tensor_copy(strided, bk[mb][:])
                for mb in range(4):
                    # SWDGE store keeps the HWDGE load FIFO unblocked
                    nc.gpsimd.dma_start(
                        out_d[
                            mb * P:(mb + 1) * P,
                            vg * 2 * G:(vg + 1) * 2 * G,
                        ],
                        outs[mb][:],
                    )
    nc.compile()
    return nc


def _get_nc():
    if "nc" not in _CACHE:
        _CACHE["nc"] = _build()
    return _CACHE["nc"]


def _dct_basis_t():
    """C^T as float32 [N, N]: C^T[i, k] = cos(pi*(2i+1)*k/(2N)).

    Matches the reference's float32 jnp computation (fp32 argument
    arithmetic) so basis rounding does not diverge from the oracle."""
    if "ct" in _CACHE:
        return _CACHE["ct"]
    ct = None
    try:
        import jax
        import jax.numpy as jnp

        cpus = jax.devices("cpu")
        with jax.default_device(cpus[0]):
            k = jnp.arange(N, dtype=jnp.float32)[:, None]
            i = jnp.arange(N, dtype=jnp.float32)[None, :]
            c = jnp.cos((jnp.pi / (2.0 * N)) * (2.0 * i + 1.0) * k)
            ct = np.ascontiguousarray(np.asarray(c).T)
    except Exception:
        pass
    if ct is None:
        k = np.arange(N, dtype=np.float32)[:, None]
        i = np.arange(N, dtype=np.float32)[None, :]
        s = math.pi / (2.0 * N)
        arg = (s * (2.0 * i + 1.0)).astype(np.float32) * k
        ct = np.ascontiguousarray(np.cos(arg.astype(np.float32)).T)
    _CACHE["ct"] = ct
    return ct


def _in_maps(x):
    x = np.asarray(x, dtype=np.float32)
    ct = _dct_basis_t()

    # host-side parity folds (exact up to fp32 rounding)
    xE = x[:H] + x[:H - 1:-1]
    xO = x[:H] - x[:H - 1:-1]
    quads = {}
    for tag, xf in (("E", xE), ("O", xO)):
        quads[tag + "E"] = np.ascontiguousarray(xf[:, :H] + xf[:, :H - 1:-1])
        quads[tag + "O"] = np.ascontiguousarray(xf[:, :H] - xf[:, :H - 1:-1])

    # parity-packed stage-2 basis: [e0|o0|e1|o1|e2|o2|e3|o3] 512-col blocks
    c1teo = np.empty((H, N), dtype=np.float32)
    for q in range(4):
        c1teo[:, 2 * q * G:(2 * q + 1) * G] = ct[:H, 2 * q * G:2 * (q + 1) * G:2]
        c1teo[:, (2 * q + 1) * G:(2 * q + 2) * G] = ct[
            :H, 2 * q * G + 1:2 * (q + 1) * G:2
        ]

    maps = []
    for c in range(NCORES):
        par = 0 if c < 4 else 1
        base = 1024 * (c % 4)
        maps.append(
            {
                "xa": quads["EE" if par == 0 else "OE"],
                "xb": quads["EO" if par == 0 else "OO"],
                "c0tp": np.ascontiguousarray(
                    ct[:H, base + par:base + 1024 + par:2]
                ),
                "c1teo": c1teo,
            }
        )
    return maps


def _assemble(results):
    full = np.empty((N, N), dtype=np.float32)
    for c in range(NCORES):
        par = 0 if c < 4 else 1
        base = 1024 * (c % 4)
        full[base + par:base + 1024 + par:2] = results[c]["out"]
    return full


def _run(x, **kwargs):
    nc = _get_nc()
    res = run_bass_kernel_spmd(
        nc, _in_maps(x), core_ids=list(range(NCORES)), **kwargs
    )
    return _assemble(res.results), res


def kernel(x):
    out, _ = _run(x)
    return out


# revision 6
# speedup vs baseline: 2.4227x; 1.1810x over previous
"""2D DCT-II (4096x4096, fp32) on 8 TRN2 NeuronCores.

out = C0 @ x @ C1^T with C0 = C1 = C, C[k, i] = cos(pi*(2i+1)*k/(2N)).

Fast-DCT folding via the basis reflection symmetries
    C[u, N-1-i]   = (-1)^u     * C[u, i]        (level 1, both axes)
    C[v, N/2-1-j] = (-1)^(v/2) * C[v, j]  (v even; level 2, column axis)

level 1 (both stages, folded on the HOST -> half FLOPs + half HBM):
  - cores 0-3 own even output rows u, cores 4-7 odd rows;
  - host supplies doubly-folded x quarters xa/xb [2048,2048] (feeding
    even-v / odd-v outputs) and basis slices;
level 2 (column axis only, even v split into v%4==0 / v%4==2):
  - xa arrives with its columns permuted [0..1023, 2047..1024] so the
    stage-1 intermediate tiles pair reflection partners at identical
    partition offsets; a 16-op DVE butterfly (running under stage 1's
    remaining matmuls) then yields the quarter-folded T2E/T2O, and the
    v%4 sections contract over only 1024 elements.

Device pipeline per core (all matmuls fp32r = full-rate FP22):
  stage 1: T(E|O)^T[j', m] = sum_i' x(a|b)[i', j'] * c0tp[i', m]
     lhsT = x tile (streamed, 1 MB DMAs), rhs = c0tp (SBUF-resident)
     -> 512 matmuls; intermediates land transposed in SBUF, exactly the
     stationary layout stage 2 needs.
  butterfly: t2e (in-place over TE') / t2o = TE'[j''] -+ TE'[j''+1024]
  stage 2: v%4==0: sum_{j''<1024} t2e^T * C[4v'', j'']     ( 64 matmuls)
           v%4==2: sum_{j''<1024} t2o^T * C[4v''+2, j'']   ( 64 matmuls)
           v odd:  sum_{j'<2048}  TO^T  * C[2v'+1, j']     (256 matmuls)
     rhs = basis (streamed), lhsT = intermediates (SBUF-resident).
  Output leaves in section-packed columns [v0 | v2 | vodd]; the host
  de-interleaves (pure numpy slicing).

PSUM: 4-bank accumulation groups alternate between two bank sets so a
group's drain (DVE/ACT copies, alternating) overlaps the next group's
matmuls. Total per-core: 896 matmuls (~203 us PE) + ~60 MB HBM.
"""

import math

import numpy as np

import concourse.mybir as mybir
import concourse.tile as tile
from concourse import bacc
from concourse.bass_utils import run_bass_kernel_spmd

N = 4096
H = N // 2  # 2048: level-1 folded contraction
Q = N // 4  # 1024: level-2 folded contraction
P = 128
HT = H // P  # 16
QT = Q // P  # 8
NCORES = 8
RB = 512  # output rows per core
G = 512  # column-group / matmul moving width
KQ = 4  # k-tiles per streaming DMA (1 MB)

f32 = mybir.dt.float32
f32r = mybir.dt.float32r

_CACHE = {}


def _build():
    nc = bacc.Bacc("TRN2", target_bir_lowering=False, debug=False)
    xa_d = nc.dram_tensor("xa", [H, H], f32r, kind="ExternalInput")
    xb_d = nc.dram_tensor("xb", [H, H], f32r, kind="ExternalInput")
    c0tp_d = nc.dram_tensor("c0tp", [H, RB], f32r, kind="ExternalInput")
    c1v02_d = nc.dram_tensor("c1v02", [Q, H], f32r, kind="ExternalInput")
    c1vo_d = nc.dram_tensor("c1vo", [H, H], f32r, kind="ExternalInput")
    out_d = nc.dram_tensor("out", [RB, N], f32, kind="ExternalOutput")

    state = {"ggc": 0}

    with tile.TileContext(nc) as tc:
        with (
            tc.tile_pool(name="persist", bufs=1) as persist,
            tc.tile_pool(name="xin", bufs=3) as xin,
            tc.tile_pool(name="cin", bufs=4) as cin,
            tc.tile_pool(name="osb", bufs=6) as osb,
            tc.tile_pool(name="ps", bufs=1, space="PSUM") as ps,
        ):
            c0tp_sb = persist.tile([P, HT, RB], f32r, tag="c0", name="c0tp_sb")
            # TE' (permuted) / TO intermediates: [j', m] as [128, 16, 512]
            t_sb = [
                persist.tile([P, HT, RB], f32r, tag=f"t{h}", name=f"t{h}_sb")
                for h in range(2)
            ]
            # level-2 odd-sign butterfly output (t2e overwrites t0 in place)
            t2o_sb = persist.tile([P, QT, RB], f32r, tag="t2o", name="t2o_sb")

            def banks(n=4):
                g = state["ggc"]
                state["ggc"] += 1
                return [
                    ps.tile(
                        [P, G], f32, tag=f"ps{(g % 2) * 4 + i}",
                        name=f"ps{(g % 2) * 4 + i}",
                    )
                    for i in range(n)
                ]

            def drain(bk, mb, dst):
                # alternate DVE/ACT so section-end drains parallelize
                if mb % 2 == 0:
                    nc.vector.tensor_copy(dst, bk[:])
                else:
                    nc.scalar.copy(dst, bk[:])

            # ---- stage 1: T(E|O)^T[j', m] = sum_i' x(a|b)[i',j'] c0tp[i',m]
            for h in range(2):
                src = xa_d if h == 0 else xb_d
                for g in range(4):  # j'-column groups of 512
                    bk = banks()
                    for kq in range(HT // KQ):
                        if h == 0 and g == 0 and kq == 0:
                            # fine-grained first chunk: first matmuls can
                            # start after ~512 KB instead of 2 MB
                            for ko in range(KQ):
                                nc.sync.dma_start(
                                    c0tp_sb[:, ko, :],
                                    c0tp_d[ko * P:(ko + 1) * P, :],
                                )
                                if ko == 0:
                                    xt = xin.tile(
                                        [P, KQ, G], f32r, tag="xt", name="xt"
                                    )
                                nc.sync.dma_start(
                                    xt[:, ko, :],
                                    src[ko * P:(ko + 1) * P, 0:G],
                                )
                        else:
                            if h == 0 and g == 0:
                                nc.sync.dma_start(
                                    c0tp_sb[:, kq * KQ:(kq + 1) * KQ, :],
                                    c0tp_d[
                                        kq * KQ * P:(kq + 1) * KQ * P, :
                                    ].rearrange("(o p) m -> p o m", p=P),
                                )
                            xt = xin.tile([P, KQ, G], f32r, tag="xt", name="xt")
                            nc.sync.dma_start(
                                xt[:],
                                src[
                                    kq * KQ * P:(kq + 1) * KQ * P,
                                    g * G:(g + 1) * G,
                                ].rearrange("(o p) n -> p o n", p=P),
                            )
                        for ko in range(KQ):
                            it = kq * KQ + ko
                            for jb in range(4):
                                nc.tensor.matmul(
                                    bk[jb][:],
                                    xt[:, ko, jb * P:(jb + 1) * P],
                                    c0tp_sb[:, it, :],
                                    start=(it == 0),
                                    stop=(it == HT - 1),
                                )
                    for jb in range(4):
                        nc.vector.tensor_copy(
                            t_sb[h][:, g * 4 + jb, :], bk[jb][:]
                        )
                if h == 0:
                    # level-2 butterfly on TE' (runs on DVE under the
                    # TO-half matmuls): t2o = lo - hi; t0[lo] += hi
                    for jt in range(QT):
                        nc.vector.tensor_tensor(
                            t2o_sb[:, jt, :],
                            t_sb[0][:, jt, :],
                            t_sb[0][:, QT + jt, :],
                            mybir.AluOpType.subtract,
                        )
                        nc.vector.tensor_tensor(
                            t_sb[0][:, jt, :],
                            t_sb[0][:, jt, :],
                            t_sb[0][:, QT + jt, :],
                            mybir.AluOpType.add,
                        )

            # ---- stage 2 ----
            # v%4==0 and v%4==2 sections: 1024-deep contraction
            for sec in range(2):  # 0: t2e (=t0[:QT]), 1: t2o
                lhs = t_sb[0] if sec == 0 else t2o_sb
                for blk in range(2):  # 512 output columns each
                    bk = banks()
                    for jq in range(QT // KQ):
                        ct = cin.tile([P, KQ, G], f32r, tag="ct", name="ct")
                        nc.sync.dma_start(
                            ct[:],
                            c1v02_d[
                                jq * KQ * P:(jq + 1) * KQ * P,
                                (2 * sec + blk) * G:(2 * sec + blk + 1) * G,
                            ].rearrange("(o p) v -> p o v", p=P),
                        )
                        for jo in range(KQ):
                            jt = jq * KQ + jo
                            for mb in range(4):
                                nc.tensor.matmul(
                                    bk[mb][:],
                                    lhs[:, jt, mb * P:(mb + 1) * P],
                                    ct[:, jo, :],
                                    start=(jt == 0),
                                    stop=(jt == QT - 1),
                                )
                    for mb in range(4):
                        ot = osb.tile([P, G], f32, tag="ot", name="ot")
                        drain(bk[mb], mb, ot[:])
                        nc.gpsimd.dma_start(
                            out_d[
                                mb * P:(mb + 1) * P,
                                (2 * sec + blk) * G:(2 * sec + blk + 1) * G,
                            ],
                            ot[:],
                        )
            # v odd section: 2048-deep contraction over TO
            for vg in range(4):  # 512 output columns each
                bk = banks()
                for jq in range(HT // KQ):
                    ct = cin.tile([P, KQ, G], f32r, tag="ct", name="ct")
                    nc.sync.dma_start(
                        ct[:],
                        c1vo_d[
                            jq * KQ * P:(jq + 1) * KQ * P,
                            vg * G:(vg + 1) * G,
                        ].rearrange("(o p) v -> p o v", p=P),
                    )
                    for jo in range(KQ):
                        jt = jq * KQ + jo
                        for mb in range(4):
                            nc.tensor.matmul(
                                bk[mb][:],
                                t_sb[1][:, jt, mb * P:(mb + 1) * P],
                                ct[:, jo, :],
                                start=(jt == 0),
                                stop=(jt == HT - 1),
                            )
                for mb in range(4):
                    ot = osb.tile([P, G], f32, tag="ot", name="ot")
                    drain(bk[mb], mb, ot[:])
                    nc.gpsimd.dma_start(
                        out_d[
                            mb * P:(mb + 1) * P,
                            2048 + vg * G:2048 + (vg + 1) * G,
                        ],
                        ot[:],
                    )
    nc.compile()
    return nc


def _get_nc():
    if "nc" not in _CACHE:
        _CACHE["nc"] = _build()
    return _CACHE["nc"]


def _dct_basis_t():
    """C^T as float32 [N, N]: C^T[i, k] = cos(pi*(2i+1)*k/(2N)).

    Matches the reference's float32 jnp computation (fp32 argument
    arithmetic) so basis rounding does not diverge from the oracle."""
    if "ct" in _CACHE:
        return _CACHE["ct"]
    ct = None
    try:
        import jax
        import jax.numpy as jnp

        cpus = jax.devices("cpu")
        with jax.default_device(cpus[0]):
            k = jnp.arange(N, dtype=jnp.float32)[:, None]
            i = jnp.arange(N, dtype=jnp.float32)[None, :]
            c = jnp.cos((jnp.pi / (2.0 * N)) * (2.0 * i + 1.0) * k)
            ct = np.ascontiguousarray(np.asarray(c).T)
    except Exception:
        pass
    if ct is None:
        k = np.arange(N, dtype=np.float32)[:, None]
        i = np.arange(N, dtype=np.float32)[None, :]
        s = math.pi / (2.0 * N)
        arg = (s * (2.0 * i + 1.0)).astype(np.float32) * k
        ct = np.ascontiguousarray(np.cos(arg.astype(np.float32)).T)
    _CACHE["ct"] = ct
    return ct


def _in_maps(x):
    x = np.asarray(x, dtype=np.float32)
    ct = _dct_basis_t()

    # level-1 host folds (exact up to fp32 rounding)
    xE = x[:H] + x[:H - 1:-1]
    xO = x[:H] - x[:H - 1:-1]
    quads = {}
    for tag, xf in (("E", xE), ("O", xO)):
        xa = xf[:, :H] + xf[:, :H - 1:-1]
        # permute xa columns [0..Q-1, H-1..Q] so stage-1 tiles align
        # level-2 reflection partners at equal partition offsets
        quads[tag + "a"] = np.ascontiguousarray(
            np.concatenate([xa[:, :Q], xa[:, :Q - 1:-1]], axis=1)
        )
        quads[tag + "b"] = np.ascontiguousarray(xf[:, :H] - xf[:, :H - 1:-1])

    # stage-2 bases
    c1v02 = np.empty((Q, H), dtype=np.float32)
    c1v02[:, :Q] = ct[:Q, 0::4]  # C[4v'', j''], j'' rows
    c1v02[:, Q:] = ct[:Q, 2::4]
    c1vo = np.ascontiguousarray(ct[:H, 1::2])

    maps = []
    for c in range(NCORES):
        par = 0 if c < 4 else 1
        base = 1024 * (c % 4)
        maps.append(
            {
                "xa": quads[("E" if par == 0 else "O") + "a"],
                "xb": quads[("E" if par == 0 else "O") + "b"],
                "c0tp": np.ascontiguousarray(
                    ct[:H, base + par:base + 1024 + par:2]
                ),
                "c1v02": c1v02,
                "c1vo": c1vo,
            }
        )
    return maps


def _assemble(results):
    full = np.empty((N, N), dtype=np.float32)
    for c in range(NCORES):
        par = 0 if c < 4 else 1
        base = 1024 * (c % 4)
        rows = full[base + par:base + 1024 + par:2]
        dev = results[c]["out"]
        rows[:, 0::4] = dev[:, 0:1024]
        rows[:, 2::4] = dev[:, 1024:2048]
        rows[:, 1::2] = dev[:, 2048:4096]
    return full


def _run(x, **kwargs):
    nc = _get_nc()
    res = run_bass_kernel_spmd(
        nc, _in_maps(x), core_ids=list(range(NCORES)), **kwargs
    )
    return _assemble(res.results), res


def kernel(x):
    out, _ = _run(x)
    return out


# revision 8
# speedup vs baseline: 2.4242x; 1.0006x over previous
"""2D DCT-II (4096x4096, fp32) on 8 TRN2 NeuronCores.

out = C0 @ x @ C1^T with C0 = C1 = C, C[k, i] = cos(pi*(2i+1)*k/(2N)).

Fast-DCT folding via the basis reflection symmetries
    C[u, N-1-i]   = (-1)^u     * C[u, i]        (level 1, both axes)
    C[v, N/2-1-j] = (-1)^(v/2) * C[v, j]  (v even; level 2, column axis)

level 1 (both stages, folded on the HOST -> half FLOPs + half HBM):
  - cores 0-3 own even output rows u, cores 4-7 odd rows;
  - host supplies doubly-folded x quarters xa/xb [2048,2048] (feeding
    even-v / odd-v outputs) and basis slices;
level 2 (column axis only, even v split into v%4==0 / v%4==2):
  - xa arrives with its columns permuted [0..1023, 2047..1024] so the
    stage-1 intermediate tiles pair reflection partners at identical
    partition offsets; a 16-op DVE butterfly (running under stage 1's
    remaining matmuls) then yields the quarter-folded T2E/T2O, and the
    v%4 sections contract over only 1024 elements.

Device pipeline per core (all matmuls fp32r = full-rate FP22):
  stage 1: T(E|O)^T[j', m] = sum_i' x(a|b)[i', j'] * c0tp[i', m]
     lhsT = x tile (streamed, 1 MB DMAs), rhs = c0tp (SBUF-resident)
     -> 512 matmuls; intermediates land transposed in SBUF, exactly the
     stationary layout stage 2 needs.
  butterfly: t2e (in-place over TE') / t2o = TE'[j''] -+ TE'[j''+1024]
  stage 2: v%4==0: sum_{j''<1024} t2e^T * C[4v'', j'']     ( 64 matmuls)
           v%4==2: sum_{j''<1024} t2o^T * C[4v''+2, j'']   ( 64 matmuls)
           v odd:  sum_{j'<2048}  TO^T  * C[2v'+1, j']     (256 matmuls)
     rhs = basis (streamed), lhsT = intermediates (SBUF-resident).
  Output leaves in section-packed columns [v0 | v2 | vodd]; the host
  de-interleaves (pure numpy slicing).

PSUM: 4-bank accumulation groups alternate between two bank sets so a
group's drain (DVE/ACT copies, alternating) overlaps the next group's
matmuls. Total per-core: 896 matmuls (~203 us PE) + ~60 MB HBM.
"""

import math

import numpy as np

import concourse.mybir as mybir
import concourse.tile as tile
from concourse import bacc
from concourse.bass_utils import run_bass_kernel_spmd

N = 4096
H = N // 2  # 2048: level-1 folded contraction
Q = N // 4  # 1024: level-2 folded contraction
P = 128
HT = H // P  # 16
QT = Q // P  # 8
NCORES = 8
RB = 512  # output rows per core
G = 512  # column-group / matmul moving width
KQ = 4  # k-tiles per streaming DMA (1 MB)

f32 = mybir.dt.float32
f32r = mybir.dt.float32r

_CACHE = {}


def _build():
    nc = bacc.Bacc("TRN2", target_bir_lowering=False, debug=False)
    xa_d = nc.dram_tensor("xa", [H, H], f32r, kind="ExternalInput")
    xb_d = nc.dram_tensor("xb", [H, H], f32r, kind="ExternalInput")
    c0tp_d = nc.dram_tensor("c0tp", [H, RB], f32r, kind="ExternalInput")
    c1v02_d = nc.dram_tensor("c1v02", [Q, H], f32r, kind="ExternalInput")
    c1vo_d = nc.dram_tensor("c1vo", [H, H], f32r, kind="ExternalInput")
    out_d = nc.dram_tensor("out", [RB, N], f32, kind="ExternalOutput")

    state = {"ggc": 0}

    with tile.TileContext(nc) as tc:
        with (
            tc.tile_pool(name="persist", bufs=1) as persist,
            tc.tile_pool(name="xin", bufs=4) as xin,
            tc.tile_pool(name="cin", bufs=4) as cin,
            tc.tile_pool(name="osb", bufs=6) as osb,
            tc.tile_pool(name="ps", bufs=1, space="PSUM") as ps,
        ):
            c0tp_sb = persist.tile([P, HT, RB], f32r, tag="c0", name="c0tp_sb")
            # TE' (permuted) / TO intermediates: [j', m] as [128, 16, 512]
            t_sb = [
                persist.tile([P, HT, RB], f32r, tag=f"t{h}", name=f"t{h}_sb")
                for h in range(2)
            ]
            # level-2 odd-sign butterfly output (t2e overwrites t0 in place)
            t2o_sb = persist.tile([P, QT, RB], f32r, tag="t2o", name="t2o_sb")

            def banks(n=4):
                g = state["ggc"]
                state["ggc"] += 1
                return [
                    ps.tile(
                        [P, G], f32, tag=f"ps{(g % 2) * 4 + i}",
                        name=f"ps{(g % 2) * 4 + i}",
                    )
                    for i in range(n)
                ]

            def drain(bk, mb, dst):
                # alternate DVE/ACT so section-end drains parallelize
                if mb % 2 == 0:
                    nc.vector.tensor_copy(dst, bk[:])
                else:
                    nc.scalar.copy(dst, bk[:])

            # PE warm-up: the HAM clock gate needs ~3.4 us of sustained
            # matmul activity to lift the PE from 1.2 to 2.4 GHz, and the
            # first real matmul can't start until ~0.5 MB of operands
            # land (~10 us incl. preamble). Chew zeros meanwhile so the
            # real stream starts warm.
            junk = persist.tile([P, P], f32, tag="junk", name="junk")
            nc.gpsimd.memset(junk[:], 0)
            jps = ps.tile([P, P], f32, tag="ps7", name="jps")
            for _ in range(32):
                nc.tensor.matmul(jps[:], junk[:], junk[:], start=True, stop=True)

            # ---- stage 1: T(E|O)^T[j', m] = sum_i' x(a|b)[i',j'] c0tp[i',m]
            for h in range(2):
                src = xa_d if h == 0 else xb_d
                for g in range(4):  # j'-column groups of 512
                    bk = banks()
                    for kq in range(HT // KQ):
                        if h == 0 and g == 0 and kq == 0:
                            # fine-grained first chunk: first matmuls can
                            # start after ~512 KB instead of 2 MB
                            for ko in range(KQ):
                                nc.scalar.dma_start(
                                    c0tp_sb[:, ko, :],
                                    c0tp_d[ko * P:(ko + 1) * P, :],
                                )
                                if ko == 0:
                                    xt = xin.tile(
                                        [P, KQ, G], f32r, tag="xt", name="xt"
                                    )
                                nc.sync.dma_start(
                                    xt[:, ko, :],
                                    src[ko * P:(ko + 1) * P, 0:G],
                                )
                        else:
                            if h == 0 and g == 0:
                                nc.scalar.dma_start(
                                    c0tp_sb[:, kq * KQ:(kq + 1) * KQ, :],
                                    c0tp_d[
                                        kq * KQ * P:(kq + 1) * KQ * P, :
                                    ].rearrange("(o p) m -> p o m", p=P),
                                )
                            xt = xin.tile([P, KQ, G], f32r, tag="xt", name="xt")
                            nc.sync.dma_start(
                                xt[:],
                                src[
                                    kq * KQ * P:(kq + 1) * KQ * P,
                                    g * G:(g + 1) * G,
                                ].rearrange("(o p) n -> p o n", p=P),
                            )
                        for ko in range(KQ):
                            it = kq * KQ + ko
                            for jb in range(4):
                                nc.tensor.matmul(
                                    bk[jb][:],
                                    xt[:, ko, jb * P:(jb + 1) * P],
                                    c0tp_sb[:, it, :],
                                    start=(it == 0),
                                    stop=(it == HT - 1),
                                )
                    for jb in range(4):
                        nc.vector.tensor_copy(
                            t_sb[h][:, g * 4 + jb, :], bk[jb][:]
                        )
                if h == 0:
                    # level-2 butterfly on TE' (runs on DVE under the
                    # TO-half matmuls): t2o = lo - hi; t0[lo] += hi
                    for jt in range(QT):
                        nc.vector.tensor_tensor(
                            t2o_sb[:, jt, :],
                            t_sb[0][:, jt, :],
                            t_sb[0][:, QT + jt, :],
                            mybir.AluOpType.subtract,
                        )
                        nc.vector.tensor_tensor(
                            t_sb[0][:, jt, :],
                            t_sb[0][:, jt, :],
                            t_sb[0][:, QT + jt, :],
                            mybir.AluOpType.add,
                        )

            # ---- stage 2 ----
            # v%4==0 and v%4==2 sections: 1024-deep contraction
            for sec in range(2):  # 0: t2e (=t0[:QT]), 1: t2o
                lhs = t_sb[0] if sec == 0 else t2o_sb
                for blk in range(2):  # 512 output columns each
                    bk = banks()
                    for jq in range(QT // KQ):
                        ct = cin.tile([P, KQ, G], f32r, tag="ct", name="ct")
                        nc.sync.dma_start(
                            ct[:],
                            c1v02_d[
                                jq * KQ * P:(jq + 1) * KQ * P,
                                (2 * sec + blk) * G:(2 * sec + blk + 1) * G,
                            ].rearrange("(o p) v -> p o v", p=P),
                        )
                        for jo in range(KQ):
                            jt = jq * KQ + jo
                            for mb in range(4):
                                nc.tensor.matmul(
                                    bk[mb][:],
                                    lhs[:, jt, mb * P:(mb + 1) * P],
                                    ct[:, jo, :],
                                    start=(jt == 0),
                                    stop=(jt == QT - 1),
                                )
                    for mb in range(4):
                        ot = osb.tile([P, G], f32, tag="ot", name="ot")
                        drain(bk[mb], mb, ot[:])
                        nc.gpsimd.dma_start(
                            out_d[
                                mb * P:(mb + 1) * P,
                                (2 * sec + blk) * G:(2 * sec + blk + 1) * G,
                            ],
                            ot[:],
                        )
            # v odd section: 2048-deep contraction over TO
            for vg in range(4):  # 512 output columns each
                bk = banks()
                for jq in range(HT // KQ):
                    ct = cin.tile([P, KQ, G], f32r, tag="ct", name="ct")
                    nc.sync.dma_start(
                        ct[:],
                        c1vo_d[
                            jq * KQ * P:(jq + 1) * KQ * P,
                            vg * G:(vg + 1) * G,
                        ].rearrange("(o p) v -> p o v", p=P),
                    )
                    for jo in range(KQ):
                        jt = jq * KQ + jo
                        for mb in range(4):
                            nc.tensor.matmul(
                                bk[mb][:],
                                t_sb[1][:, jt, mb * P:(mb + 1) * P],
                                ct[:, jo, :],
                                start=(jt == 0),
                                stop=(jt == HT - 1),
                            )
                for mb in range(4):
                    ot = osb.tile([P, G], f32, tag="ot", name="ot")
                    drain(bk[mb], mb, ot[:])
                    eng = nc.sync if vg == 3 else nc.gpsimd
                    eng.dma_start(
                        out_d[
                            mb * P:(mb + 1) * P,
                            2048 + vg * G:2048 + (vg + 1) * G,
                        ],
                        ot[:],
                    )
    nc.compile()
    return nc


def _get_nc():
    if "nc" not in _CACHE:
        _CACHE["nc"] = _build()
    return _CACHE["nc"]


def _dct_basis_t():
    """C^T as float32 [N, N]: C^T[i, k] = cos(pi*(2i+1)*k/(2N)).

    Matches the reference's float32 jnp computation (fp32 argument
    arithmetic) so basis rounding does not diverge from the oracle."""
    if "ct" in _CACHE:
        return _CACHE["ct"]
    ct = None
    try:
        import jax
        import jax.numpy as jnp

        cpus = jax.devices("cpu")
        with jax.default_device(cpus[0]):
            k = jnp.arange(N, dtype=jnp.float32)[:, None]
            i = jnp.arange(N, dtype=jnp.float32)[None, :]
            c = jnp.cos((jnp.pi / (2.0 * N)) * (2.0 * i + 1.0) * k)
            ct = np.ascontiguousarray(np.asarray(c).T)
    except Exception:
        pass
    if ct is None:
        k = np.arange(N, dtype=np.float32)[:, None]
        i = np.arange(N, dtype=np.float32)[None, :]
        s = math.pi / (2.0 * N)
        arg = (s * (2.0 * i + 1.0)).astype(np.float32) * k
        ct = np.ascontiguousarray(np.cos(arg.astype(np.float32)).T)
    _CACHE["ct"] = ct
    return ct


def _in_maps(x):
    x = np.asarray(x, dtype=np.float32)
    ct = _dct_basis_t()

    # level-1 host folds (exact up to fp32 rounding)
    xE = x[:H] + x[:H - 1:-1]
    xO = x[:H] - x[:H - 1:-1]
    quads = {}
    for tag, xf in (("E", xE), ("O", xO)):
        xa = xf[:, :H] + xf[:, :H - 1:-1]
        # permute xa columns [0..Q-1, H-1..Q] so stage-1 tiles align
        # level-2 reflection partners at equal partition offsets
        quads[tag + "a"] = np.ascontiguousarray(
            np.concatenate([xa[:, :Q], xa[:, :Q - 1:-1]], axis=1)
        )
        quads[tag + "b"] = np.ascontiguousarray(xf[:, :H] - xf[:, :H - 1:-1])

    # stage-2 bases
    c1v02 = np.empty((Q, H), dtype=np.float32)
    c1v02[:, :Q] = ct[:Q, 0::4]  # C[4v'', j''], j'' rows
    c1v02[:, Q:] = ct[:Q, 2::4]
    c1vo = np.ascontiguousarray(ct[:H, 1::2])

    maps = []
    for c in range(NCORES):
        par = 0 if c < 4 else 1
        base = 1024 * (c % 4)
        maps.append(
            {
                "xa": quads[("E" if par == 0 else "O") + "a"],
                "xb": quads[("E" if par == 0 else "O") + "b"],
                "c0tp": np.ascontiguousarray(
                    ct[:H, base + par:base + 1024 + par:2]
                ),
                "c1v02": c1v02,
                "c1vo": c1vo,
            }
        )
    return maps


def _assemble(results):
    full = np.empty((N, N), dtype=np.float32)
    for c in range(NCORES):
        par = 0 if c < 4 else 1
        base = 1024 * (c % 4)
        rows = full[base + par:base + 1024 + par:2]
        dev = results[c]["out"]
        rows[:, 0::4] = dev[:, 0:1024]
        rows[:, 2::4] = dev[:, 1024:2048]
        rows[:, 1::2] = dev[:, 2048:4096]
    return full


def _run(x, **kwargs):
    nc = _get_nc()
    res = run_bass_kernel_spmd(
        nc, _in_maps(x), core_ids=list(range(NCORES)), **kwargs
    )
    return _assemble(res.results), res


def kernel(x):
    out, _ = _run(x)
    return out
